# revision 19
# baseline (speedup 1.0000x reference)
"""CoAttentionLayer kernel for 8 Trainium2 NeuronCores.

Reference computes (per batch b):
    qkv = x @ W_qkv  -> q,k,v heads [H=16, L=2048, D=64]
    s1 = q1 @ k2^T * scale ; o1 = s1 @ v2   (NO softmax -> purely linear)
    s2 = q2 @ k1^T * scale ; o2 = s2 @ v1
    out = concat(o1, o2) @ W_proj + b_proj

Because there is no softmax, associativity collapses the attention:
    o1 = q1 @ M2,  M2_h = scale * k2_h^T @ v2_h          ([64,64] per head)
    out_half1 = q1_flat @ G1,  G1 rows (h,d) = (M2_h @ Wp_h)[d,:]
so the [1024x1024] score matrices never exist. Total ~71 GFLOP.

Sharding: 8 cores = 4 batches x 2 head-groups (8 heads each). Each core
computes a partial projection output for its batch; host sums the two
head-group partials per batch and adds b_proj.

Matmul inputs are float16 (fp32 PSUM accumulation): fp16 runs the PE at
full rate with the fast (pipelined) weight-load path, and its 11-bit
mantissa keeps the end-to-end relative error at ~6e-4, essentially the
same as float32r's FP22 multiplies on this data (all intermediate
tensors stay < 60 in magnitude, far from fp16 range limits).
Measured ~175-190 us per iteration on 8 cores (data-parallel over
B=4 x 2 head-groups), vs ~1.3 ms for a naive implementation.
"""

import numpy as np

import concourse.bass as bass
import concourse.tile as tile
from concourse import bacc, mybir
from concourse import bass_utils

F32 = mybir.dt.float32
import os as _os
_PEONLY = bool(_os.environ.get("KERNEL_PEONLY"))
_DT = _os.environ.get("KERNEL_DTYPE", "fp16")
if _DT == "bf16":
    F32R = mybir.dt.bfloat16
    _IN_NP = "bfloat16"
elif _DT == "f32r":
    F32R = mybir.dt.float32r
    _IN_NP = "float32"
else:
    F32R = mybir.dt.float16       # fp16: bf16-speed weight path, 10-bit mantissa
    _IN_NP = "float16"

P = 128          # SBUF partitions
L = 2048         # sequence length
HALF = 1024      # coatten split point
C = 1024         # model dim
HG = 512         # per-core head-group width (8 heads x 64)
NCI = C // P     # 8 contraction chunks for the qkv projection
NT = 512         # matmul moving free dim (one PSUM bank of fp32)
SCALE = 64 ** -0.5

N_CORES = 8


def _r(ap):
    """Matmul-input tiles are float32r already; no-op kept for clarity."""
    return ap


def _build_core_program(tc, nc, xT, wq, wk, wv, wp, out):
    """Emit the per-core Tile program.

    DRAM inputs (per core, partition-major layouts prepared by the host):
      xT  [128, 8, 2048]  x[b].T   chunked: xT[p, c, i] = x[b, i, c*128+p]
      wq/wk/wv [128, 8, 512]       w[p, c, n] = W[c*128+p, n]
      wp  [128, 4, 1024]           wp[p, c, n] = W_proj[g*512 + c*128+p, n]
    DRAM output:
      out [2048, 1024]  partial (this head-group's contribution)
    """
    with (
        tc.tile_pool(name="wconst", bufs=1) as wpool,
        tc.tile_pool(name="persist", bufs=1) as ppool,
        tc.tile_pool(name="psum_mm", bufs=6, space="PSUM") as psum_pool,
        tc.tile_pool(name="psum_mt", bufs=2, space="PSUM") as psum_mt_pool,
        tc.tile_pool(name="ostage", bufs=4) as opool,
        tc.tile_pool(name="xpool", bufs=12) as xpool,
    ):
        # ---- weight loads: per-chunk DMAs interleaved with the first
        # half's xT chunks so the first matmul groups unblock early ----
        wq_sb = wpool.tile([P, NCI, HG], F32R)
        wk_sb = wpool.tile([P, NCI, HG], F32R)
        wv_sb = wpool.tile([P, NCI, HG], F32R)
        wp_sb = wpool.tile([P, 4, C], F32R)

        # per-chunk xT tiles, 12-slot rotation so half-2 chunks prefetch
        # while half-1 is still computing
        xt_tiles = {}

        def load_x_chunk(hf, ci):
            t = xpool.tile([P, HALF], F32R, tag="xc", name=f"xc{hf}_{ci}")
            nc.sync.dma_start(
                t, xT[:, ci, hf * HALF:(hf + 1) * HALF])
            xt_tiles[(hf, ci)] = t

        for ci in range(NCI):
            load_x_chunk(0, ci)
            nc.sync.dma_start(wq_sb[:, ci, :], wq[:, ci, :])
        for ci in range(NCI):
            nc.sync.dma_start(wk_sb[:, ci, :], wk[:, ci, :])
        for ci in range(NCI):
            nc.sync.dma_start(wv_sb[:, ci, :], wv[:, ci, :])
        for wc in range(4):
            nc.sync.dma_start(wp_sb[:, wc, :], wp[:, wc, :])

        # persistent across phases
        qT_sb = ppool.tile([P, 2, 4, HALF], F32R)  # [p, half, hd_chunk, i]
        # block-diagonal per head-pair: mt_sb[:, hf, m] = diag(Mt_2m, Mt_2m+1)
        mt_sb = ppool.tile([P, 2, 4, P], F32R)
        nc.any.memset(mt_sb[:, :, :, :].bitcast(F32), 0.0)

        with (
            tc.tile_pool(name="kvpool", bufs=1) as kvpool,
        ):
            for hf in range(2):
                if hf == 1:
                    for ci in range(NCI):
                        load_x_chunk(1, ci)

                k_sb = kvpool.tile([P, 8, HG], F32R, tag="k")
                v_sb = kvpool.tile([P, 8, HG], F32R, tag="v")

                # qT[hd, i] = sum_c wq[c, hd] * x[i, c]  (transposed q)
                for hc in range(4):
                    for it in range(2):
                        ps = psum_pool.tile([P, NT], F32, tag="ps")
                        for ci in range(NCI):
                            nc.tensor.matmul(
                                ps,
                                _r(wq_sb[:, ci, hc * P:(hc + 1) * P]),
                                _r(xt_tiles[(hf, ci)][:, it * NT:(it + 1) * NT]),
                                start=(ci == 0), stop=(ci == NCI - 1))
                        if not _PEONLY:
                            nc.vector.tensor_copy(
                                qT_sb[:, hf, hc, it * NT:(it + 1) * NT], ps)

                # k, v in natural layout [i, hd]
                for w_sb, dst in ((wk_sb, k_sb), (wv_sb, v_sb)):
                    for ib in range(8):
                        ps = psum_pool.tile([P, NT], F32, tag="ps")
                        for ci in range(NCI):
                            nc.tensor.matmul(
                                ps,
                                _r(xt_tiles[(hf, ci)][:, ib * P:(ib + 1) * P]),
                                _r(w_sb[:, ci, :]),
                                start=(ci == 0), stop=(ci == NCI - 1))
                        if not _PEONLY:
                            nc.vector.tensor_copy(dst[:, ib, :], ps)

                # Mt = scale * v^T @ k   ([c2, c1]; diag 64x64 blocks = M_h^T)
                # keep only the per-head diagonal blocks, stored
                # block-diagonally per head pair for full-K G matmuls
                # per head-pair: psum [128, 128] = v_pair^T @ k_pair; only
                # the two diagonal 64x64 blocks (per-head Mt) are kept
                for mb in range(4):
                    ps = psum_mt_pool.tile([P, P], F32, tag="ps_mt")
                    for jb in range(8):
                        lhs = (xt_tiles[(hf, jb)][:, mb * P:(mb + 1) * P]
                               if _PEONLY else v_sb[:, jb, mb * P:(mb + 1) * P])
                        rhs = (xt_tiles[(hf, jb)][:, 0:P]
                               if _PEONLY else k_sb[:, jb, mb * P:(mb + 1) * P])
                        nc.tensor.matmul(
                            ps, _r(lhs), _r(rhs),
                            start=(jb == 0), stop=(jb == 7))
                    if not _PEONLY:
                        for sub in range(2):
                            pr = slice(sub * 64, sub * 64 + 64)
                            nc.scalar.mul(
                                mt_sb[pr, hf, mb, sub * 64:sub * 64 + 64],
                                ps[pr, sub * 64:sub * 64 + 64], SCALE)

        with tc.tile_pool(name="gpool", bufs=1) as gpool:
            # G rows (h*64+d1) = (M_h @ Wp_h)[d1, :]; lhsT = M_h^T = Mt_h
            g_sb = gpool.tile([P, 2, 4, C], F32R)  # [p, out_half, row_chunk, n]
            for ho in range(2):
                src = 1 - ho  # out half 1 uses M from sequence half 2
                for hp in range(4):          # head pair
                    for nt_i in range(2):
                        ps = psum_pool.tile([P, NT], F32, tag="ps")
                        nc.tensor.matmul(
                            ps,
                            _r(wp_sb[:, hp, 0:P] if _PEONLY
                               else mt_sb[:, src, hp, :]),
                            _r(wp_sb[:, hp, nt_i * NT:(nt_i + 1) * NT]),
                            start=True, stop=True)
                        if not _PEONLY:
                            nc.vector.tensor_copy(
                                g_sb[:, ho, hp, nt_i * NT:(nt_i + 1) * NT], ps)

            # out_half = q_half @ G_half
            for ho in range(2):
                for ib in range(8):
                    for nt_i in range(2):
                        ps = psum_pool.tile([P, NT], F32, tag="ps")
                        for hc in range(4):
                            lhs = (wq_sb[:, hc, 0:P] if _PEONLY
                                   else qT_sb[:, ho, hc, ib * P:(ib + 1) * P])
                            rhs = (wp_sb[:, hc, nt_i * NT:(nt_i + 1) * NT]
                                   if _PEONLY
                                   else g_sb[:, ho, hc, nt_i * NT:(nt_i + 1) * NT])
                            nc.tensor.matmul(
                                ps, _r(lhs), _r(rhs),
                                start=(hc == 0), stop=(hc == 3))
                        if not _PEONLY:
                            ot = opool.tile([P, NT], F32, tag="ot")
                            if (ib % 3) == 2:
                                nc.scalar.copy(ot, ps)
                            else:
                                nc.vector.tensor_copy(ot, ps)
                            nc.sync.dma_start(
                                out[ho * HALF + ib * P: ho * HALF + (ib + 1) * P,
                                    nt_i * NT:(nt_i + 1) * NT],
                                ot)


def build_nc(reps=1):
    nc = bacc.Bacc("TRN2", target_bir_lowering=False, debug=False,
                   enable_asserts=False, num_devices=N_CORES)
    xT = nc.dram_tensor("xT", [P, NCI, L], F32R, kind="ExternalInput").ap()
    wq = nc.dram_tensor("wq", [P, NCI, HG], F32R, kind="ExternalInput").ap()
    wk = nc.dram_tensor("wk", [P, NCI, HG], F32R, kind="ExternalInput").ap()
    wv = nc.dram_tensor("wv", [P, NCI, HG], F32R, kind="ExternalInput").ap()
    wp = nc.dram_tensor("wp", [P, 4, C], F32R, kind="ExternalInput").ap()
    out = nc.dram_tensor("out_p", [L, C], F32, kind="ExternalOutput").ap()

    with tile.TileContext(nc) as tc:
        if reps == 1:
            _build_core_program(tc, nc, xT, wq, wk, wv, wp, out)
        else:
            with tc.For_i(0, reps, 1, hint_engines=(
                    mybir.EngineType.PE, mybir.EngineType.DVE,
                    mybir.EngineType.Activation, mybir.EngineType.SP)):
                _build_core_program(tc, nc, xT, wq, wk, wv, wp, out)
    nc.compile()
    return nc


_NC_CACHE = None


def _get_nc():
    global _NC_CACHE
    if _NC_CACHE is None:
        _NC_CACHE = build_nc()
    return _NC_CACHE


def _part_major(a, nchunks):
    """[nchunks*128, N] -> contiguous [128, nchunks, N]."""
    n = a.shape[1]
    a = a.reshape(nchunks, P, n).transpose(1, 0, 2)
    if _IN_NP == "bfloat16":
        import ml_dtypes
        a = a.astype(ml_dtypes.bfloat16)
    elif _IN_NP == "float16":
        a = a.astype(np.float16)
    return np.ascontiguousarray(a)


def make_in_maps(x, W_qkv, W_proj):
    in_maps = []
    for c in range(N_CORES):
        b, g = c // 2, c % 2
        xT = np.ascontiguousarray(x[b].T)          # [1024, 2048]
        in_maps.append({
            "xT": _part_major(xT, NCI),
            "wq": _part_major(
                np.ascontiguousarray(W_qkv[:, g * HG:(g + 1) * HG]), NCI),
            "wk": _part_major(
                np.ascontiguousarray(W_qkv[:, C + g * HG:C + (g + 1) * HG]),
                NCI),
            "wv": _part_major(
                np.ascontiguousarray(
                    W_qkv[:, 2 * C + g * HG:2 * C + (g + 1) * HG]), NCI),
            "wp": _part_major(
                np.ascontiguousarray(W_proj[g * HG:(g + 1) * HG, :]), 4),
        })
    return in_maps


def kernel(x, W_qkv, W_proj, b_proj, coatten, _trace=False):
    x = np.asarray(x, dtype=np.float32)
    W_qkv = np.asarray(W_qkv, dtype=np.float32)
    W_proj = np.asarray(W_proj, dtype=np.float32)
    b_proj = np.asarray(b_proj, dtype=np.float32)
    assert int(coatten) == HALF, f"kernel hardcodes coatten=1024, got {coatten}"
    B = x.shape[0]
    assert x.shape == (4, L, C) and W_qkv.shape == (C, 3 * C)

    nc = _get_nc()
    in_maps = make_in_maps(x, W_qkv, W_proj)
    res = bass_utils.run_bass_kernel_spmd(
        nc, in_maps, core_ids=list(range(N_CORES)), trace=_trace)
    parts = [r["out_p"] for r in res.results]
    out = np.stack([parts[2 * b] + parts[2 * b + 1] for b in range(B)])
    out = out + b_proj[None, None, :]
    if _trace:
        return out.astype(np.float32), res
    return out.astype(np.float32)
